# revision 2
# baseline (speedup 1.0000x reference)
"""Multi-head attention (B=2, T=2048, C=1024, H=16, D=64) on 8 TRN2 cores.

Sharding: core c = 4*b + g handles batch b (2-way data parallel) and head
group g (4 heads, 4-way tensor parallel). qkv is column-parallel, proj is
row-parallel; the 4 partial proj outputs per batch are summed on host.

Device kernel (per core), all matmuls in bf16 with fp32 PSUM accumulate:
  qT = wq.T @ xT          [256, 2048]   (head dims on partitions)
  kT = wk.T @ xT          [256, 2048]
  v  = xT.T @ wv          [2048, 4, 65] (ones column appended per head)
  per head h, per 1024-wide query chunk:
    for each 128-wide key tile tk:
      scoresT = kT_h[:,tk].T @ qT_h     [128, 1024]  (keys on partitions)
      expT    = exp(scoresT / 8)        bf16
      pav    += vhat_h[tk].T @ expT     [65, 1024]   (row 64 = softmax denom)
    recip denom -> DRAM -> broadcast over 64 partitions -> attn_hT = num * r
  y = sum_h attn_hT.T @ wp_h            [2048, 1024] fp32 partial out
"""
import sys
import numpy as np

sys.path.insert(0, "/opt/trn_rl_repo")
import ml_dtypes

B, T, C = 2, 2048, 1024
NH, HD = 16, 64
HG = 4                    # heads per core
GC = HG * HD              # 256 columns per core
KT = C // 128             # 8 k-tiles for qkv contraction
TT = T // 128             # 16 token tiles
QC = 2                    # query chunks of 1024
QW = T // QC              # 1024
NCORES = 8

_cache = {}


def _build():
    import concourse.bass as bass
    import concourse.mybir as mybir
    import concourse.tile as tile
    from concourse import bacc

    f32 = mybir.dt.float32
    bf16 = mybir.dt.bfloat16

    nc = bacc.Bacc(None, target_bir_lowering=False)

    xt = nc.dram_tensor("xt", [C, T], bf16, kind="ExternalInput")
    wq = nc.dram_tensor("wq", [C, GC], bf16, kind="ExternalInput")
    wk = nc.dram_tensor("wk", [C, GC], bf16, kind="ExternalInput")
    wv = nc.dram_tensor("wv", [C, GC], bf16, kind="ExternalInput")
    wp = nc.dram_tensor("wp", [GC, C], bf16, kind="ExternalInput")
    bq = nc.dram_tensor("bq", [128, 2], f32, kind="ExternalInput")
    bk = nc.dram_tensor("bk", [128, 2], f32, kind="ExternalInput")
    bv = nc.dram_tensor("bv", [1, GC], f32, kind="ExternalInput")
    y = nc.dram_tensor("y", [T, C], f32, kind="ExternalOutput")

    with tile.TileContext(nc) as tc:
        with (
            tc.tile_pool(name="ins", bufs=1) as ins,
            tc.tile_pool(name="big", bufs=1) as bigp,
            tc.tile_pool(name="work", bufs=3) as work,
            tc.tile_pool(name="numsb", bufs=2) as numsb,
            tc.tile_pool(name="ps", bufs=2, space="PSUM") as ps,
            tc.tile_pool(name="psav", bufs=1, space="PSUM") as psav,
            tc.tile_pool(name="dram", bufs=8, space="DRAM") as dpool,
        ):
            # ---- input staging ----
            xt_sb = ins.tile([128, KT, T], bf16, tag="xt")
            for kt in range(KT):
                nc.sync.dma_start(xt_sb[:, kt, :], xt[kt * 128:(kt + 1) * 128, :])
            wq_sb = ins.tile([128, KT, GC], bf16, tag="wq")
            wk_sb = ins.tile([128, KT, GC], bf16, tag="wk")
            wv_sb = ins.tile([128, KT, GC], bf16, tag="wv")
            nc.sync.dma_start(wq_sb[:], wq.rearrange("(a p) n -> p a n", p=128))
            nc.sync.dma_start(wk_sb[:], wk.rearrange("(a p) n -> p a n", p=128))
            nc.sync.dma_start(wv_sb[:], wv.rearrange("(a p) n -> p a n", p=128))
            wp_sb = ins.tile([64, HG, C], bf16, tag="wp")
            nc.sync.dma_start(wp_sb[:], wp.rearrange("(h p) n -> p h n", p=64))
            bq_sb = ins.tile([128, 2], f32, tag="bq")
            bk_sb = ins.tile([128, 2], f32, tag="bk")
            nc.sync.dma_start(bq_sb[:], bq[:])
            nc.sync.dma_start(bk_sb[:], bk[:])
            bv_sb = ins.tile([128, GC], f32, tag="bv")
            nc.gpsimd.dma_start(bv_sb[:], bv[0:1, :].to_broadcast([128, GC]))

            # ---- qkv projections ----
            vhat_sb = bigp.tile([128, TT, HG, HD + 1], bf16, tag="vhat")
            nc.vector.memset(vhat_sb[:, :, :, HD:HD + 1], 1.0)
            for tt in range(TT):
                pv = ps.tile([128, 1024], f32, tag="sc")
                for kt in range(KT):
                    nc.tensor.matmul(
                        pv[:, 0:GC],
                        xt_sb[:, kt, tt * 128:(tt + 1) * 128],
                        wv_sb[:, kt, :],
                        start=(kt == 0), stop=(kt == KT - 1),
                    )
                nc.vector.tensor_add(
                    vhat_sb[:, tt, :, 0:HD], pv[:, 0:GC], bv_sb[:])

            qt_sb = bigp.tile([128, 2, T], bf16, tag="qt")
            kt_sb = bigp.tile([128, 2, T], bf16, tag="kt")
            for mt in range(2):
                for src_sb, dst_sb, bias_sb in (
                    (wq_sb, qt_sb, bq_sb), (wk_sb, kt_sb, bk_sb)):
                    for ts in range(4):
                        pq = ps.tile([128, 1024], f32, tag="sc")
                        for kt in range(KT):
                            nc.tensor.matmul(
                                pq[:, 0:512],
                                src_sb[:, kt, mt * 128:(mt + 1) * 128],
                                xt_sb[:, kt, ts * 512:(ts + 1) * 512],
                                start=(kt == 0), stop=(kt == KT - 1),
                            )
                        nc.vector.tensor_scalar_add(
                            dst_sb[:, mt, ts * 512:(ts + 1) * 512],
                            pq[:, 0:512], bias_sb[:, mt:mt + 1])

            # ---- attention ----
            attn_sb = bigp.tile([64, HG, T], bf16, tag="attn")
            for qc in range(QC):
                q0 = qc * QW
                for h in range(HG):
                    mt, off = h // 2, (h % 2) * 64
                    pav = psav.tile([65, QW], f32, tag="av")
                    for tk in range(TT):
                        psc = ps.tile([128, 1024], f32, tag="sc")
                        for half in range(2):
                            nc.tensor.matmul(
                                psc[:, half * 512:(half + 1) * 512],
                                kt_sb[off:off + 64, mt, tk * 128:(tk + 1) * 128],
                                qt_sb[off:off + 64, mt,
                                      q0 + half * 512:q0 + (half + 1) * 512],
                                start=True, stop=True,
                            )
                        et = work.tile([128, QW], bf16, tag="expt")
                        nc.scalar.activation(
                            et[:], psc[:], mybir.ActivationFunctionType.Exp,
                            bias=0.0, scale=0.125)
                        for half in range(2):
                            nc.tensor.matmul(
                                pav[:, half * 512:(half + 1) * 512],
                                vhat_sb[:, tk, h, :],
                                et[:, half * 512:(half + 1) * 512],
                                start=(tk == 0), stop=(tk == TT - 1),
                            )
                    num = numsb.tile([65, QW], f32, tag="num")
                    nc.vector.tensor_copy(num[:], pav[:])
                    nc.vector.reciprocal(num[64:65, :], num[64:65, :])
                    dscr = dpool.tile([1, QW], f32, tag="den")
                    nc.sync.dma_start(dscr[:], num[64:65, :])
                    rbc = numsb.tile([64, QW], f32, tag="rbc")
                    nc.gpsimd.dma_start(rbc[:], dscr[0:1, :].to_broadcast([64, QW]))
                    nc.vector.tensor_mul(
                        attn_sb[:, h, q0:q0 + QW], num[0:64, :], rbc[:])

            # ---- output projection (row-parallel partial) ----
            for tt in range(TT):
                for ns in range(2):
                    py = ps.tile([128, 1024], f32, tag="sc")
                    for h in range(HG):
                        nc.tensor.matmul(
                            py[:, 0:512],
                            attn_sb[:, h, tt * 128:(tt + 1) * 128],
                            wp_sb[:, h, ns * 512:(ns + 1) * 512],
                            start=(h == 0), stop=(h == HG - 1),
                        )
                    ysb = work.tile([128, 512], f32, tag="ysb")
                    nc.vector.tensor_copy(ysb[:], py[:, 0:512])
                    nc.sync.dma_start(
                        y[tt * 128:(tt + 1) * 128, ns * 512:(ns + 1) * 512],
                        ysb[:])

    nc.compile()
    return nc


def _get_nc():
    if "nc" not in _cache:
        _cache["nc"] = _build()
    return _cache["nc"]


def make_in_maps(x, w_qkv, b_qkv, w_proj):
    bf = ml_dtypes.bfloat16
    x = np.asarray(x, dtype=np.float32)
    w_qkv = np.asarray(w_qkv, dtype=np.float32)
    b_qkv = np.asarray(b_qkv, dtype=np.float32)
    in_maps = []
    for c in range(NCORES):
        b, g = divmod(c, HG)
        cols = slice(g * GC, (g + 1) * GC)
        in_maps.append({
            "xt": np.ascontiguousarray(x[b].T).astype(bf),
            "wq": np.ascontiguousarray(w_qkv[:, 0 * C:1 * C][:, cols]).astype(bf),
            "wk": np.ascontiguousarray(w_qkv[:, 1 * C:2 * C][:, cols]).astype(bf),
            "wv": np.ascontiguousarray(w_qkv[:, 2 * C:3 * C][:, cols]).astype(bf),
            "wp": np.ascontiguousarray(
                np.asarray(w_proj, dtype=np.float32)[g * GC:(g + 1) * GC, :]
            ).astype(bf),
            "bq": np.ascontiguousarray(
                b_qkv[0 * C:1 * C][cols].reshape(2, 128).T).astype(np.float32),
            "bk": np.ascontiguousarray(
                b_qkv[1 * C:2 * C][cols].reshape(2, 128).T).astype(np.float32),
            "bv": np.ascontiguousarray(
                b_qkv[2 * C:3 * C][cols].reshape(1, GC)).astype(np.float32),
        })
    return in_maps


def gather(results, b_proj):
    b_proj = np.asarray(b_proj, dtype=np.float32)
    out = np.zeros((B, T, C), dtype=np.float32)
    for c in range(NCORES):
        b = c // HG
        out[b] += results[c]["y"]
    out += b_proj[None, None, :]
    return out


def kernel(x, w_qkv, b_qkv, w_proj, b_proj, _trace=False, _tmpdir=None):
    from concourse import bass_utils
    nc = _get_nc()
    in_maps = make_in_maps(x, w_qkv, b_qkv, w_proj)
    res = bass_utils.run_bass_kernel_spmd(
        nc, in_maps, core_ids=list(range(NCORES)), trace=_trace,
        tmpdir=_tmpdir)
    _cache["last_result"] = res
    return gather(res.results, b_proj)


# revision 8
# speedup vs baseline: 1.0132x; 1.0132x over previous
"""Multi-head attention (B=2, T=2048, C=1024, H=16, D=64) on 8 TRN2 cores.

Sharding: core c = 4*b + g handles batch b (2-way data parallel) and head
group g (4 heads, 4-way tensor parallel). qkv is column-parallel, proj is
row-parallel; the 4 partial proj outputs per batch are summed on host.

Device kernel (per core), all matmuls in bf16 with fp32 PSUM accumulate:
  qT = wq.T @ xT          [256, 2048]   (head dims on partitions)
  kT = wk.T @ xT          [256, 2048]
  v  = xT.T @ wv          [2048, 4, 65] (ones column appended per head)
  per head h, per 1024-wide query chunk:
    for each 128-wide key tile tk:
      scoresT = kT_h[:,tk].T @ qT_h     [128, 1024]  (keys on partitions)
      expT    = exp(scoresT / 8)        bf16
      pav    += vhat_h[tk].T @ expT     [65, 1024]   (row 64 = softmax denom)
    recip denom -> DRAM -> broadcast over 64 partitions -> attn_hT = num * r
  y = sum_h attn_hT.T @ wp_h            [2048, 1024] fp32 partial out
"""
import sys
import numpy as np

sys.path.insert(0, "/opt/trn_rl_repo")
import ml_dtypes

B, T, C = 2, 2048, 1024
NH, HD = 16, 64
HG = 4                    # heads per core
GC = HG * HD              # 256 columns per core
KT = C // 128             # 8 k-tiles for qkv contraction
TT = T // 128             # 16 token tiles
QC = 2                    # query chunks of 1024
QW = T // QC              # 1024
NCORES = 8

_cache = {}


def _enable_ldw_opt():
    """walrus --enable-ldw-opt=false serializes LDWEIGHTS with MATMUL
    (~376ns/MM instead of ~216). Flip it on."""
    from concourse import bass_utils
    if getattr(bass_utils, "_ldw_patched", False):
        return
    orig = bass_utils.run_command

    def patched(argv, **kw):
        argv = ["--enable-ldw-opt=true" if a == "--enable-ldw-opt=false" else a
                for a in argv]
        return orig(argv, **kw)

    bass_utils.run_command = patched
    bass_utils._ldw_patched = True


def _dedup_ldweights(nc):
    """Drop InstLdweights identical to the immediately-previous PE weight
    load (only matmuls between), moving its waits onto the next matmul.
    The PE array keeps stationary weights across matmuls, so the reload is
    pure overhead (~107ns serialized, walrus ldw-opt is disabled)."""
    import concourse.mybir as mybir
    removed = 0
    for f in nc.m.functions:
        for bb in f.blocks:
            out = []
            prev_key = None
            pending = []
            for inst in bb.instructions:
                tn = type(inst).__name__
                if tn == "InstLdweights":
                    key = (str(inst.ins[0]), str(inst.is_transpose),
                           str(inst.perf_mode), str(inst.tile_position))
                    si = inst.sync_info
                    if key == prev_key and not (si and si.on_update):
                        if si:
                            pending.extend(si.on_wait)
                        removed += 1
                        continue
                    prev_key = key
                elif tn in ("InstMatmult", "InstMatmultMx"):
                    if getattr(inst, "is_transpose", False):
                        prev_key = None
                    if pending:
                        si = inst.sync_info
                        inst.sync_info = mybir.SyncInfo(
                            on_wait=(list(si.on_wait) if si else []) + pending,
                            on_update=(list(si.on_update) if si else []))
                        pending = []
                elif tn in ("InstUnconditionalBranch", "InstCall",
                            "InstCompareBranch"):
                    prev_key = None
                out.append(inst)
            assert not pending
            bb.instructions[:] = out
    return removed


def _build():
    import concourse.bass as bass
    import concourse.mybir as mybir
    import concourse.tile as tile
    from concourse import bacc

    f32 = mybir.dt.float32
    bf16 = mybir.dt.bfloat16

    nc = bacc.Bacc(None, target_bir_lowering=False)

    xt = nc.dram_tensor("xt", [C, T], bf16, kind="ExternalInput")
    wq = nc.dram_tensor("wq", [C, GC], bf16, kind="ExternalInput")
    wk = nc.dram_tensor("wk", [C, GC], bf16, kind="ExternalInput")
    wv = nc.dram_tensor("wv", [C, GC], bf16, kind="ExternalInput")
    wp = nc.dram_tensor("wp", [GC, C], bf16, kind="ExternalInput")
    bq = nc.dram_tensor("bq", [128, 2], f32, kind="ExternalInput")
    bk = nc.dram_tensor("bk", [128, 2], f32, kind="ExternalInput")
    bv = nc.dram_tensor("bv", [1, GC], f32, kind="ExternalInput")
    y = nc.dram_tensor("y", [T, C], f32, kind="ExternalOutput")

    with tile.TileContext(nc) as tc:
        with (
            tc.tile_pool(name="ins", bufs=1) as ins,
            tc.tile_pool(name="big", bufs=1) as bigp,
            tc.tile_pool(name="work", bufs=3) as work,
            tc.tile_pool(name="numsb", bufs=2) as numsb,
            tc.tile_pool(name="ps", bufs=2, space="PSUM") as ps,
            tc.tile_pool(name="psav", bufs=1, space="PSUM") as psav,
            tc.tile_pool(name="dram", bufs=8, space="DRAM") as dpool,
        ):
            # ---- input staging ----
            wq_sb = ins.tile([128, KT, GC], bf16, tag="wq")
            wk_sb = ins.tile([128, KT, GC], bf16, tag="wk")
            wv_sb = ins.tile([128, KT, GC], bf16, tag="wv")
            nc.sync.dma_start(wq_sb[:], wq.rearrange("(a p) n -> p a n", p=128))
            nc.sync.dma_start(wk_sb[:], wk.rearrange("(a p) n -> p a n", p=128))
            nc.sync.dma_start(wv_sb[:], wv.rearrange("(a p) n -> p a n", p=128))
            wp_sb = ins.tile([64, HG, C], bf16, tag="wp")
            nc.sync.dma_start(wp_sb[:], wp.rearrange("(h p) n -> p h n", p=64))
            bq_sb = ins.tile([128, 2], f32, tag="bq")
            bk_sb = ins.tile([128, 2], f32, tag="bk")
            nc.sync.dma_start(bq_sb[:], bq[:])
            nc.sync.dma_start(bk_sb[:], bk[:])
            bv_sb = ins.tile([128, GC], f32, tag="bv")
            nc.gpsimd.dma_start(bv_sb[:], bv[0:1, :].to_broadcast([128, GC]))
            # xt in 16 chunks so the first q/k matmuls start ~1us in and the
            # PE warms up (HAM) instead of stalling on one 4MB transfer
            xt_sb = ins.tile([128, KT, T], bf16, tag="xt")
            for kt in range(KT):
                for half in range(2):
                    nc.sync.dma_start(
                        xt_sb[:, kt, half * 1024:(half + 1) * 1024],
                        xt[kt * 128:(kt + 1) * 128,
                           half * 1024:(half + 1) * 1024])

            # ---- q/k projections (mt=0 first so attention h0/h1 can start;
            #      2 MMs per kt share one weight load -> ldw dedup) ----
            qt_sb = bigp.tile([128, 2, T], bf16, tag="qt")
            kt_sb = bigp.tile([128, 2, T], bf16, tag="kt")

            def qk_group(mt):
                for src_sb, dst_sb, bias_sb in (
                    (wq_sb, qt_sb, bq_sb), (wk_sb, kt_sb, bk_sb)):
                    for tsp in range(2):
                        pq = ps.tile([128, 1024], f32, tag="sc")
                        for kt in range(KT):
                            for half in range(2):
                                nc.tensor.matmul(
                                    pq[:, half * 512:(half + 1) * 512],
                                    src_sb[:, kt, mt * 128:(mt + 1) * 128],
                                    xt_sb[:, kt,
                                          tsp * 1024 + half * 512:
                                          tsp * 1024 + (half + 1) * 512],
                                    start=(kt == 0), stop=(kt == KT - 1),
                                )
                        nc.vector.tensor_scalar_add(
                            dst_sb[:, mt, tsp * 1024:(tsp + 1) * 1024],
                            pq[:], bias_sb[:, mt:mt + 1])

            qk_group(0)

            # ---- v projection (+ ones column for the softmax denominator) ----
            vhat_sb = bigp.tile([128, TT, HG, HD + 1], bf16, tag="vhat")
            nc.vector.memset(vhat_sb[:, :, :, HD:HD + 1], 1.0)
            for tt in range(TT):
                pv = ps.tile([128, 1024], f32, tag="sc")
                for kt in range(KT):
                    nc.tensor.matmul(
                        pv[:, 0:GC],
                        xt_sb[:, kt, tt * 128:(tt + 1) * 128],
                        wv_sb[:, kt, :],
                        start=(kt == 0), stop=(kt == KT - 1),
                    )
                nc.vector.tensor_add(
                    vhat_sb[:, tt, :, 0:HD], pv[:, 0:GC], bv_sb[:])

            qk_group(1)

            # ---- attention ----
            attn_sb = bigp.tile([64, HG, T], bf16, tag="attn")
            for qc in range(QC):
                q0 = qc * QW
                for h in range(HG):
                    mt, off = h // 2, (h % 2) * 64
                    pav = psav.tile([65, QW], f32, tag="av")
                    for tk in range(TT):
                        psc = ps.tile([128, 1024], f32, tag="sc")
                        for half in range(2):
                            nc.tensor.matmul(
                                psc[:, half * 512:(half + 1) * 512],
                                kt_sb[off:off + 64, mt, tk * 128:(tk + 1) * 128],
                                qt_sb[off:off + 64, mt,
                                      q0 + half * 512:q0 + (half + 1) * 512],
                                start=True, stop=True,
                            )
                        et = work.tile([128, QW], bf16, tag="expt")
                        nc.scalar.activation(
                            et[:], psc[:], mybir.ActivationFunctionType.Exp,
                            bias=0.0, scale=0.125)
                        for half in range(2):
                            nc.tensor.matmul(
                                pav[:, half * 512:(half + 1) * 512],
                                vhat_sb[:, tk, h, :],
                                et[:, half * 512:(half + 1) * 512],
                                start=(tk == 0), stop=(tk == TT - 1),
                            )
                    num = numsb.tile([65, QW], f32, tag="num")
                    nc.vector.tensor_copy(num[:], pav[:])
                    nc.vector.reciprocal(num[64:65, :], num[64:65, :])
                    dscr = dpool.tile([1, QW], f32, tag="den")
                    nc.sync.dma_start(dscr[:], num[64:65, :])
                    rbc = numsb.tile([64, QW], f32, tag="rbc")
                    nc.gpsimd.dma_start(rbc[:], dscr[0:1, :].to_broadcast([64, QW]))
                    nc.vector.tensor_mul(
                        attn_sb[:, h, q0:q0 + QW], num[0:64, :], rbc[:])

            # ---- output projection (row-parallel partial); both ns halves
            #      accumulate in one psum tile so the per-h weight load is
            #      shared by 2 matmuls ----
            for tt in range(TT):
                py = ps.tile([128, 1024], f32, tag="sc")
                for h in range(HG):
                    for ns in range(2):
                        nc.tensor.matmul(
                            py[:, ns * 512:(ns + 1) * 512],
                            attn_sb[:, h, tt * 128:(tt + 1) * 128],
                            wp_sb[:, h, ns * 512:(ns + 1) * 512],
                            start=(h == 0), stop=(h == HG - 1),
                        )
                ysb = work.tile([128, 1024], f32, tag="ysb")
                nc.vector.tensor_copy(ysb[:], py[:])
                nc.sync.dma_start(y[tt * 128:(tt + 1) * 128, :], ysb[:])

    nc.compile()
    n = _dedup_ldweights(nc)
    import logging
    logging.getLogger(__name__).info("dedup_ldweights removed %d", n)
    return nc


def _get_nc():
    if "nc" not in _cache:
        _cache["nc"] = _build()
    return _cache["nc"]


def make_in_maps(x, w_qkv, b_qkv, w_proj):
    bf = ml_dtypes.bfloat16
    x = np.asarray(x, dtype=np.float32)
    w_qkv = np.asarray(w_qkv, dtype=np.float32)
    b_qkv = np.asarray(b_qkv, dtype=np.float32)
    in_maps = []
    for c in range(NCORES):
        b, g = divmod(c, HG)
        cols = slice(g * GC, (g + 1) * GC)
        in_maps.append({
            "xt": np.ascontiguousarray(x[b].T).astype(bf),
            "wq": np.ascontiguousarray(w_qkv[:, 0 * C:1 * C][:, cols]).astype(bf),
            "wk": np.ascontiguousarray(w_qkv[:, 1 * C:2 * C][:, cols]).astype(bf),
            "wv": np.ascontiguousarray(w_qkv[:, 2 * C:3 * C][:, cols]).astype(bf),
            "wp": np.ascontiguousarray(
                np.asarray(w_proj, dtype=np.float32)[g * GC:(g + 1) * GC, :]
            ).astype(bf),
            "bq": np.ascontiguousarray(
                b_qkv[0 * C:1 * C][cols].reshape(2, 128).T).astype(np.float32),
            "bk": np.ascontiguousarray(
                b_qkv[1 * C:2 * C][cols].reshape(2, 128).T).astype(np.float32),
            "bv": np.ascontiguousarray(
                b_qkv[2 * C:3 * C][cols].reshape(1, GC)).astype(np.float32),
        })
    return in_maps


def gather(results, b_proj):
    b_proj = np.asarray(b_proj, dtype=np.float32)
    out = np.zeros((B, T, C), dtype=np.float32)
    for c in range(NCORES):
        b = c // HG
        out[b] += results[c]["y"]
    out += b_proj[None, None, :]
    return out


def kernel(x, w_qkv, b_qkv, w_proj, b_proj, _trace=False, _tmpdir=None):
    from concourse import bass_utils
    nc = _get_nc()
    in_maps = make_in_maps(x, w_qkv, b_qkv, w_proj)
    res = bass_utils.run_bass_kernel_spmd(
        nc, in_maps, core_ids=list(range(NCORES)), trace=_trace,
        tmpdir=_tmpdir)
    _cache["last_result"] = res
    return gather(res.results, b_proj)
